# revision 1
# baseline (speedup 1.0000x reference)
"""Softmax-weighted nearest-neighbor aggregation (DiffusionStar) on 8 TRN2 cores.

Strategy:
  - Shard the train set (N=50000) across 8 cores (6250 rows each, padded to 6272).
  - Per core, two-phase softmax (both phases DMA-bound, ~38 MB each):
      Phase 1: scores s[b,n] = (2/a_b)<x_b, t_n> - ||t_n||^2 via fp16 GEMM
               (contraction over d, streaming a host-pretransposed fp16 train
               slice); the ||t||^2 row is subtracted in fp32 on DVE and the
               per-group row-max tracked.
      Phase 2: p = exp(g'*(s - M)) on ACT (fp16 out, g' = a^2/(2(1-acp)),
               per-partition scale/bias, fused running sum), p transposed on
               PE, then ACC = p @ train (fp16 GEMM streaming the natural-
               layout slice, fp32 PSUM accumulate across all 49 n-chunks).
               Phase-2 DMA prefetch overlaps the phase boundary.
  - Host merges (M, S, ACC) across cores with the standard online-softmax
    combine and applies the final coefficients in fp64.

Numerics: train/x are cast to fp16 for the GEMMs; products accumulate in fp32
PSUM; ||t||^2 is applied in fp32. Validated end-to-end error ~5e-4 (abs-max
relative) vs the fp32 reference; softmax argmax is preserved.
Measured: ~247-272 us HW exec time on 8 axon-tunneled TRN2 cores.
"""

import numpy as np

B = 64
D = 3072
N = 50000
NCORES = 8
N_LOC = N // NCORES          # 6250
N_PAD = 6272                 # 49 * 128
KD = D // 128                # 24
KN = N_PAD // 128            # 49
DJ = D // 512                # 6
GROUPS = [(i * 512, 512) for i in range(12)] + [(6144, 128)]
PAD_TRSQ = 1e9
NAT_BUFS = 8
INTERLEAVE_P2 = True   # interleave transpose/DMA/GEMM2 per chunk
FUSED_TTR = False      # tensor_tensor_reduce reading PSUM crashes TRN2 HW

_CACHED = {}


def _build_nc():
    import concourse.bacc as bacc
    import concourse.tile as tile
    from concourse import mybir
    from contextlib import ExitStack

    f16 = mybir.dt.float16
    f32 = mybir.dt.float32

    nc = bacc.Bacc("TRN2", target_bir_lowering=False, debug=False)

    tT = nc.dram_tensor("tT", [D, N_PAD], f16, kind="ExternalInput").ap()
    tn = nc.dram_tensor("tn", [N_PAD, D], f16, kind="ExternalInput").ap()
    xT = nc.dram_tensor("xT", [D, B], f16, kind="ExternalInput").ap()
    ident = nc.dram_tensor("ident", [B, B], f16, kind="ExternalInput").ap()
    trsq = nc.dram_tensor("trsq", [B, N_PAD], f32, kind="ExternalInput").ap()
    gcol = nc.dram_tensor("gcol", [B, 1], f32, kind="ExternalInput").ap()

    acc_out = nc.dram_tensor("acc_out", [B, D], f32, kind="ExternalOutput").ap()
    s_out = nc.dram_tensor("s_out", [B, 1], f32, kind="ExternalOutput").ap()
    m_out = nc.dram_tensor("m_out", [B, 1], f32, kind="ExternalOutput").ap()

    NG = len(GROUPS)

    with tile.TileContext(nc) as tc, ExitStack() as ctx:
        const = ctx.enter_context(tc.tile_pool(name="const", bufs=1))
        kTp = ctx.enter_context(tc.tile_pool(name="kT", bufs=2))
        natp = ctx.enter_context(tc.tile_pool(name="nat", bufs=NAT_BUFS))
        sb = ctx.enter_context(tc.tile_pool(name="sb", bufs=1))

        # --- constants ---
        xT_sb = const.tile([128, KD, B], f16)
        nc.sync.dma_start(xT_sb[:], xT.rearrange("(k p) b -> p k b", p=128))
        id_sb = const.tile([B, B], f16)
        nc.sync.dma_start(id_sb[:], ident[:])
        trsq_sb = const.tile([B, N_PAD], f32)
        nc.sync.dma_start(trsq_sb[:], trsq[:])
        g_sb = const.tile([B, 1], f32)
        nc.sync.dma_start(g_sb[:], gcol[:])

        mpart = sb.tile([B, NG], f32)
        ssum = sb.tile([B, NG], f32)
        stat = sb.tile([B, 4], f32)
        acc_sb = sb.tile([B, D], f32)
        sc_tiles = []
        p_tiles = []

        tTr = tT.rearrange("(k p) n -> p k n", p=128)  # [128, KD, N_PAD]

        # --- phase 2 DMA prefetch happens naturally via pool bufs; issue
        #     phase-1 pipeline first ---
        with tc.tile_pool(name="psS", bufs=2, space="PSUM") as psS:
            for gi, (n0, W) in enumerate(GROUPS):
                kT = kTp.tile([128, KD, 512], f16, tag="kT")
                nc.sync.dma_start(kT[:, :, :W], tTr[:, :, n0:n0 + W])
                ps = psS.tile([B, 512], f32, tag="ps")
                for k in range(KD):
                    nc.tensor.matmul(ps[:, :W], xT_sb[:, k, :], kT[:, k, :W],
                                     start=(k == 0), stop=(k == KD - 1))
                sc = sb.tile([B, 512], f32, tag=f"sc{gi}")
                sc_tiles.append(sc)
                if FUSED_TTR:
                    # sc = ps - trsq ; mpart[gi] = rowmax(sc)
                    nc.vector.tensor_tensor_reduce(
                        out=sc[:, :W], in0=ps[:, :W],
                        in1=trsq_sb[:, n0:n0 + W], scale=1.0, scalar=-1e38,
                        op0=mybir.AluOpType.subtract, op1=mybir.AluOpType.max,
                        accum_out=mpart[:, gi:gi + 1])
                else:
                    nc.vector.tensor_tensor(sc[:, :W], ps[:, :W],
                                            trsq_sb[:, n0:n0 + W],
                                            op=mybir.AluOpType.subtract)
                    nc.vector.reduce_max(mpart[:, gi:gi + 1], sc[:, :W],
                                         axis=mybir.AxisListType.X)

        # --- global max, bias = -g*M ---
        nc.vector.reduce_max(stat[:, 0:1], mpart[:, :NG],
                             axis=mybir.AxisListType.X)
        nc.vector.tensor_tensor(stat[:, 2:3], g_sb[:], stat[:, 0:1],
                                op=mybir.AluOpType.mult)
        nc.vector.tensor_scalar_mul(stat[:, 2:3], stat[:, 2:3], -1.0)

        # --- exp -> transpose -> GEMM2, pipelined per group ---
        with tc.tile_pool(name="psT", bufs=2, space="PSUM") as psT, \
             tc.tile_pool(name="psA", bufs=1, space="PSUM") as psA:
            acc_ps = psA.tile([B, DJ, 512], f32)
            pT_tiles = []
            if INTERLEAVE_P2:
                for gi, (n0, W) in enumerate(GROUPS):
                    p = sb.tile([B, 512], f16, tag=f"p{gi}")
                    p_tiles.append(p)
                    nc.scalar.activation(p[:, :W], sc_tiles[gi][:, :W],
                                         mybir.ActivationFunctionType.Exp,
                                         bias=stat[:, 2:3], scale=g_sb[:],
                                         accum_out=ssum[:, gi:gi + 1])
                    for ci in range(W // 128):
                        c = n0 // 128 + ci
                        pt_ps = psT.tile([128, B], f16, tag="pt")
                        nc.tensor.transpose(pt_ps[:],
                                            p[:, ci * 128:(ci + 1) * 128],
                                            id_sb[:])
                        pT = sb.tile([128, B], f16, tag=f"pT{c}")
                        nc.vector.tensor_copy(pT[:], pt_ps[:])
                        nat = natp.tile([128, D], f16, tag="nat")
                        nc.sync.dma_start(nat[:], tn[c * 128:(c + 1) * 128, :])
                        for j in range(DJ):
                            nc.tensor.matmul(acc_ps[:, j, :], pT[:],
                                             nat[:, j * 512:(j + 1) * 512],
                                             start=(c == 0), stop=(c == KN - 1))
            else:
                for gi, (n0, W) in enumerate(GROUPS):
                    p = sb.tile([B, 512], f16, tag=f"p{gi}")
                    p_tiles.append(p)
                    nc.scalar.activation(p[:, :W], sc_tiles[gi][:, :W],
                                         mybir.ActivationFunctionType.Exp,
                                         bias=stat[:, 2:3], scale=g_sb[:],
                                         accum_out=ssum[:, gi:gi + 1])
                for c in range(KN):
                    gi = c // 4
                    ci = c % 4
                    pt_ps = psT.tile([128, B], f16, tag="pt")
                    nc.tensor.transpose(pt_ps[:],
                                        p_tiles[gi][:, ci * 128:(ci + 1) * 128],
                                        id_sb[:])
                    pT = sb.tile([128, B], f16, tag=f"pT{c}")
                    nc.vector.tensor_copy(pT[:], pt_ps[:])
                    pT_tiles.append(pT)
                for c in range(KN):
                    nat = natp.tile([128, D], f16, tag="nat")
                    nc.sync.dma_start(nat[:], tn[c * 128:(c + 1) * 128, :])
                    for j in range(DJ):
                        nc.tensor.matmul(acc_ps[:, j, :], pT_tiles[c][:],
                                         nat[:, j * 512:(j + 1) * 512],
                                         start=(c == 0), stop=(c == KN - 1))
            for j in range(DJ):
                nc.scalar.copy(acc_sb[:, j * 512:(j + 1) * 512],
                               acc_ps[:, j, :])
                nc.sync.dma_start(acc_out[:, j * 512:(j + 1) * 512],
                                  acc_sb[:, j * 512:(j + 1) * 512])

        nc.vector.reduce_sum(stat[:, 1:2], ssum[:, :NG],
                             axis=mybir.AxisListType.X)
        nc.sync.dma_start(s_out[:], stat[:, 1:2])
        nc.sync.dma_start(m_out[:], stat[:, 0:1])

    nc.compile()
    return nc


def _get_nc():
    if "nc" not in _CACHED:
        _CACHED["nc"] = _build_nc()
    return _CACHED["nc"]


def kernel(x, train, alphas_cumprod, t, **_unused):
    from concourse.bass_utils import run_bass_kernel_spmd

    x = np.asarray(x)
    train = np.asarray(train)
    alphas_cumprod = np.asarray(alphas_cumprod)
    t = np.asarray(t).astype(np.int64)

    xf = x.reshape(B, -1).astype(np.float32)
    tf = train.reshape(N, -1).astype(np.float32)

    acp_t = alphas_cumprod.astype(np.float64)[t]
    a = np.sqrt(acp_t)
    om = 1.0 - acp_t
    gp32 = (a * a / (2.0 * om)).astype(np.float32)   # softmax scale on s''
    xscale = (2.0 / a).astype(np.float32)            # fold into x

    trsq_full = np.einsum("nd,nd->n", tf.astype(np.float64),
                          tf.astype(np.float64)).astype(np.float32)

    t16 = tf.astype(np.float16)
    x16T = np.ascontiguousarray(
        (xscale[:, None] * xf).astype(np.float16).T)  # [D, B]
    ident = np.eye(B, dtype=np.float16)
    g_col = gp32.reshape(B, 1)

    in_maps = []
    for c in range(NCORES):
        sl = slice(c * N_LOC, (c + 1) * N_LOC)
        nat = np.zeros((N_PAD, D), dtype=np.float16)
        nat[:N_LOC] = t16[sl]
        tTc = np.zeros((D, N_PAD), dtype=np.float16)
        tTc[:, :N_LOC] = t16[sl].T
        trsq_c = np.full((N_PAD,), PAD_TRSQ, dtype=np.float32)
        trsq_c[:N_LOC] = trsq_full[sl]
        trsq_c = np.ascontiguousarray(
            np.broadcast_to(trsq_c[None, :], (B, N_PAD)))
        in_maps.append({
            "tT": tTc,
            "tn": nat,
            "xT": x16T,
            "ident": ident,
            "trsq": trsq_c,
            "gcol": g_col,
        })

    nc = _get_nc()
    res = run_bass_kernel_spmd(nc, in_maps, list(range(NCORES)))
    _CACHED["last_results"] = res

    # --- host-side online-softmax merge across cores (fp64) ---
    g64 = gp32.astype(np.float64)
    Ms = np.stack([res.results[c]["m_out"][:, 0].astype(np.float64)
                   for c in range(NCORES)])          # [C, B]
    Ss = np.stack([res.results[c]["s_out"][:, 0].astype(np.float64)
                   for c in range(NCORES)])          # [C, B]
    ACCs = np.stack([res.results[c]["acc_out"].astype(np.float64)
                     for c in range(NCORES)])        # [C, B, D]
    Mg = Ms.max(axis=0)                              # [B]
    scale = np.exp(g64[None, :] * (Ms - Mg[None, :]))  # [C, B]
    den = (scale * Ss).sum(axis=0)                   # [B]
    num = (scale[:, :, None] * ACCs).sum(axis=0)     # [B, D]
    weighted = num / den[:, None]

    coef_x = 1.0 / np.sqrt(om)
    coef_x_hat = a / np.sqrt(om)
    out = coef_x[:, None] * xf.astype(np.float64) - coef_x_hat[:, None] * weighted
    return out.reshape(x.shape).astype(np.float32)

